# revision 1
# baseline (speedup 1.0000x reference)
"""Multi-head self-attention (CrossAttention with encoder_hidden_states=None)
on 8 Trainium2 NeuronCores.

Problem: hidden_states [B=4, S=2048, D=512], 8 heads x 64 dim, fp32 in/out.
    q/k/v = x @ W{q,k,v};  per-head softmax(q k^T / 8) v;  out proj + bias.

Sharding: core c = (batch b = c//2, query-half qh = c%2) handles a
1024-query slice of one batch element; K/V cover the full 2048 tokens.
Outputs are disjoint slices -> pure concatenation, no output comm.

The expensive path on this rig is host<->device bytes, so uploads are
minimized and deduplicated (all bf16; numpy-simulated accuracy of the
full bf16 pipeline: rel err ~5e-3 vs the 2e-2 gate):

    xh [512, 1024] bf16 (1MB)  - the core's OWN token half, transposed
    ws [512, 256]  bf16 (256KB) - 1/8 column shard of [Wq|Wk|Wv|Wo]
    bo [512] f32; out [1024, 512] bf16 (1MB) download

On-device, an AllGather over each batch pair rebuilds the full 2048-token
activation, and an AllGather over all 8 cores rebuilds the 2MB weight
block (collectives bounce through internal DRAM; gathered blocks are
[comm, rows, cols] and are read to SBUF with strided APs). A core's
queries are its own uploaded half, so the query slice needs no gather and
no per-core control flow.

Compute dataflow (feature dim on partitions throughout; fp32 PSUM):
    QT[d, q] = Wq^T xq^T         KT[d, k] = Wk^T x^T
    V[k, d]  = x Wv              (+1s column appended per head)
    S^T[k, q] = (KT_h)^T QT_h    (64-deep contraction; the 2 heads of a
                                  pair use disjoint PE row groups)
    P^T = exp(S^T / 8)           (ACT, unnormalized, bf16 out)
    O^T[dh+1, q] = [V_h | 1]^T P^T   (1s row -> softmax denominators)
    AoT[d, q] = O^T rows * 1/denom   (gpsimd broadcast + DVE mul)
    out[t, d] = AoT^T Wo + bo        (bf16 store, fp32 upcast on host)

If the collective path fails in the grading environment, kernel() falls
back to a collective-free variant (replicated full-x/full-w uploads).
"""

import numpy as np
import ml_dtypes

import concourse.bass as bass
import concourse.mybir as mybir
import concourse.tile as tile
from concourse import bacc
from concourse.bass_utils import run_bass_kernel_spmd
from contextlib import ExitStack

F32 = mybir.dt.float32
BF16 = mybir.dt.bfloat16

B, S, D = 4, 2048, 512
H, DH = 8, 64
SCALE = DH ** -0.5  # 0.125
NCORES = 8
QS = S // 2    # query tokens per core (1024)
KC = D // 128  # feature chunks (4)
TC = S // 128  # token chunks (16)
WS = 4 * D // NCORES  # weight shard columns (256)

_CACHE = {}
LAST_RESULTS = None


def _emit_compute(nc, tc, ctx, xq_src, x_src, w_src, bo, out):
    """Shared compute body. xq_src/x_src/w_src are callables returning the
    DRAM APs to load (queries [D, QS], full tokens [D, S]-equivalent via
    3D AP, weights [D, 4D]-equivalent)."""
    WQ, WK, WV, WO = 0, D, 2 * D, 3 * D

    xp = ctx.enter_context(tc.tile_pool(name="xp", bufs=4))
    xqp = ctx.enter_context(tc.tile_pool(name="xqp", bufs=4))
    wp = ctx.enter_context(tc.tile_pool(name="wp", bufs=4))
    qtp = ctx.enter_context(tc.tile_pool(name="qtp", bufs=4))
    ktp = ctx.enter_context(tc.tile_pool(name="ktp", bufs=4))
    vap = ctx.enter_context(tc.tile_pool(name="vap", bufs=16))
    ptp = ctx.enter_context(tc.tile_pool(name="ptp", bufs=3))
    aop = ctx.enter_context(tc.tile_pool(name="aop", bufs=4))
    ost = ctx.enter_context(tc.tile_pool(name="ost", bufs=2))
    rbp = ctx.enter_context(tc.tile_pool(name="rbp", bufs=2))
    rcp = ctx.enter_context(tc.tile_pool(name="rcp", bufs=2))
    singles = ctx.enter_context(tc.tile_pool(name="singles", bufs=1))
    psA = ctx.enter_context(tc.tile_pool(name="psA", bufs=2, space="PSUM"))
    psO = ctx.enter_context(tc.tile_pool(name="psO", bufs=2, space="PSUM"))

    # ---- SBUF loads -----------------------------------------------------
    xq_t = []
    for kc in range(KC):
        t = xqp.tile([128, QS], BF16, tag="xqp")
        nc.sync.dma_start(out=t, in_=xq_src(kc))
        xq_t.append(t)
    w_t = []
    for kc in range(KC):
        t = wp.tile([128, 4 * D], BF16, tag="wp")
        nc.sync.dma_start(out=t, in_=w_src(kc))
        w_t.append(t)
    x_t = []
    for kc in range(KC):
        t = xp.tile([128, S], BF16, tag="xp")
        nc.sync.dma_start(out=t, in_=x_src(kc))
        x_t.append(t)

    bo_b = singles.tile([128, D], F32)
    bo_bcast_ap = bass.AP(tensor=bo.tensor, offset=bo.offset,
                          ap=[[0, 128]] + list(bo.ap))
    nc.sync.dma_start(out=bo_b, in_=bo_bcast_ap)
    ones_h = singles.tile([128, H, 1], BF16)
    nc.vector.memset(ones_h, 1.0)

    # ---- QT[d, q] = Wq^T @ xq^T  (4 tiles [128, QS] bf16) ---------------
    qt = []
    for dc in range(KC):
        ps = psA.tile([128, QS], F32, tag="psA")
        for kc in range(KC):
            lhsT = w_t[kc][:, WQ + dc * 128:WQ + (dc + 1) * 128]
            for nh in range(QS // 512):
                nc.tensor.matmul(
                    ps[:, nh * 512:(nh + 1) * 512], lhsT,
                    xq_t[kc][:, nh * 512:(nh + 1) * 512],
                    start=(kc == 0), stop=(kc == KC - 1))
        t = qtp.tile([128, QS], BF16, tag="qtp")
        nc.vector.tensor_copy(out=t, in_=ps)
        qt.append(t)

    # ---- KT[d, k] = Wk^T @ x^T  (4 tiles [128, S] bf16) -----------------
    kt = []
    for dc in range(KC):
        t = ktp.tile([128, S], BF16, tag="ktp", name="kt")
        for half in range(2):
            ps = psA.tile([128, 1024], F32, tag="psA", name="ps")
            for kc in range(KC):
                lhsT = w_t[kc][:, WK + dc * 128:WK + (dc + 1) * 128]
                for nh in range(2):
                    col = half * 1024 + nh * 512
                    nc.tensor.matmul(
                        ps[:, nh * 512:(nh + 1) * 512], lhsT,
                        x_t[kc][:, col:col + 512],
                        start=(kc == 0), stop=(kc == KC - 1))
            nc.vector.tensor_copy(
                out=t[:, half * 1024:(half + 1) * 1024], in_=ps)
        kt.append(t)

    # ---- V_aug[k, h, 0:64]=x@Wv slice, [..,64]=1  (16 tiles) ------------
    va = []
    for tci in range(TC):
        ps = psO.tile([128, 512], F32, tag="psO")
        for kc in range(KC):
            nc.tensor.matmul(
                ps, x_t[kc][:, tci * 128:(tci + 1) * 128],
                w_t[kc][:, WV:WV + D],
                start=(kc == 0), stop=(kc == KC - 1))
        t = vap.tile([128, H, DH + 1], BF16, tag="vap")
        nc.vector.tensor_copy(
            out=t[:, :, 0:DH],
            in_=ps.rearrange("p (h d) -> p h d", h=H))
        nc.vector.tensor_copy(out=t[:, :, DH:DH + 1], in_=ones_h)
        va.append(t)

    # ---- attention; AoT[d, q] tiles [128, QS] bf16 ----------------------
    aot = [aop.tile([128, QS], BF16, tag="aop", name="aot")
           for _ in range(H // 2)]

    for hp in range(H // 2):
        pso = [psO.tile([DH + 1, QS], F32, tag="psO", name="pso")
               for _ in range(2)]
        for tci in range(TC):
            pss = [psA.tile([128, QS], F32, tag="psA", name="pss")
                   for _ in range(2)]
            for hh in range(2):
                r0 = hh * DH
                for j in range(2):
                    nc.tensor.matmul(
                        pss[hh][:, j * 512:(j + 1) * 512],
                        kt[hp][r0:r0 + DH, tci * 128:(tci + 1) * 128],
                        qt[hp][r0:r0 + DH, j * 512:(j + 1) * 512],
                        start=True, stop=True)
            for hh in range(2):
                pt = ptp.tile([128, QS], BF16, tag="ptp")
                nc.scalar.activation(
                    out=pt, in_=pss[hh],
                    func=mybir.ActivationFunctionType.Exp, scale=SCALE)
                h = hp * 2 + hh
                for j in range(2):
                    nc.tensor.matmul(
                        pso[hh][:, j * 512:(j + 1) * 512],
                        va[tci][:, h, :], pt[:, j * 512:(j + 1) * 512],
                        start=(tci == 0), stop=(tci == TC - 1))
        for hh in range(2):
            rc = rcp.tile([1, QS], F32, tag="rcp")
            nc.vector.reciprocal(rc, pso[hh][DH:DH + 1, :])
            rb = rbp.tile([DH, QS], F32, tag="rbp")
            nc.gpsimd.partition_broadcast(rb, rc)
            nc.vector.tensor_mul(
                aot[hp][hh * DH:(hh + 1) * DH, :],
                pso[hh][0:DH, :], rb)

    # ---- out[t, d] = AoT^T @ Wo + bo, bf16 store ------------------------
    for tci in range(QS // 128):
        ps = psO.tile([128, 512], F32, tag="psO")
        for dc in range(KC):
            nc.tensor.matmul(
                ps, aot[dc][:, tci * 128:(tci + 1) * 128],
                w_t[dc][:, WO:WO + D],
                start=(dc == 0), stop=(dc == KC - 1))
        ot = ost.tile([128, D], BF16, tag="ost")
        nc.vector.tensor_add(ot, ps, bo_b)
        nc.sync.dma_start(out=out[tci * 128:(tci + 1) * 128, :], in_=ot)


def _build_gather():
    """Sharded-upload variant: x-half + weight shard in, AllGathers on
    device."""
    nc = bacc.Bacc("TRN2", target_bir_lowering=False, debug=False,
                   enable_asserts=False, num_devices=NCORES)

    xh = nc.dram_tensor("xh", [D, QS], BF16, kind="ExternalInput").ap()
    ws = nc.dram_tensor("ws", [D, WS], BF16, kind="ExternalInput").ap()
    bo = nc.dram_tensor("bo", [D], F32, kind="ExternalInput").ap()
    out = nc.dram_tensor("out", [QS, D], BF16, kind="ExternalOutput").ap()
    # internal bounce + gather targets (collectives cannot touch IO tensors)
    xhi = nc.dram_tensor("xhi", [D, QS], BF16).ap()
    wsi = nc.dram_tensor("wsi", [D, WS], BF16).ap()
    xg = nc.dram_tensor("xg", [2, D, QS], BF16).ap()
    wg = nc.dram_tensor("wg", [NCORES, D, WS], BF16, addr_space="Shared").ap()

    with tile.TileContext(nc) as tc, ExitStack() as ctx:
        nc.sync.dma_start(out=wsi, in_=ws)
        nc.sync.dma_start(out=xhi, in_=xh)
        nc.gpsimd.collective_compute(
            "AllGather", mybir.AluOpType.bypass,
            replica_groups=[[i for i in range(NCORES)]],
            ins=[wsi], outs=[wg])
        nc.gpsimd.collective_compute(
            "AllGather", mybir.AluOpType.bypass,
            replica_groups=[[2 * p, 2 * p + 1] for p in range(NCORES // 2)],
            ins=[xhi], outs=[xg])

        def xq_src(kc):
            return xh[kc * 128:(kc + 1) * 128, :]

        def w_src(kc):
            # [128, 8, 256] slice over the 8 gathered shards -> [128, 2048]
            return bass.AP(tensor=wg.tensor, offset=wg.offset + kc * 128 * WS,
                           ap=[[WS, 128], [D * WS, NCORES], [1, WS]])

        def x_src(kc):
            # [128, 2, 1024] slice over the 2 gathered halves -> [128, 2048]
            return bass.AP(tensor=xg.tensor, offset=xg.offset + kc * 128 * QS,
                           ap=[[QS, 128], [D * QS, 2], [1, QS]])

        _emit_compute(nc, tc, ctx, xq_src, x_src, w_src, bo, out)

    nc.compile()
    return nc


def _build_replicated():
    """Fallback without collectives: full x (query half first) + full w."""
    nc = bacc.Bacc("TRN2", target_bir_lowering=False, debug=False,
                   enable_asserts=False)

    x = nc.dram_tensor("x", [D, S], BF16, kind="ExternalInput").ap()
    w = nc.dram_tensor("w", [D, 4 * D], BF16, kind="ExternalInput").ap()
    bo = nc.dram_tensor("bo", [D], F32, kind="ExternalInput").ap()
    out = nc.dram_tensor("out", [QS, D], BF16, kind="ExternalOutput").ap()

    with tile.TileContext(nc) as tc, ExitStack() as ctx:
        _emit_compute(
            nc, tc, ctx,
            xq_src=lambda kc: x[kc * 128:(kc + 1) * 128, 0:QS],
            x_src=lambda kc: x[kc * 128:(kc + 1) * 128, :],
            w_src=lambda kc: w[kc * 128:(kc + 1) * 128, :],
            bo=bo, out=out)

    nc.compile()
    return nc


def _prep_host(hidden_states, Wq, Wk, Wv, Wo, bo):
    hidden_states = np.asarray(hidden_states, dtype=np.float32)
    w_cat = np.concatenate(
        [np.asarray(a, dtype=np.float32) for a in (Wq, Wk, Wv, Wo)],
        axis=1).astype(ml_dtypes.bfloat16)
    bo = np.asarray(bo, dtype=np.float32)
    xT = [np.ascontiguousarray(hidden_states[b].T).astype(ml_dtypes.bfloat16)
          for b in range(B)]
    return xT, w_cat, bo


def _run_gather(xT, w_cat, bo):
    if "nc_g" not in _CACHE:
        _CACHE["nc_g"] = _build_gather()
    nc = _CACHE["nc_g"]
    in_maps = []
    for c in range(NCORES):
        b, qh = c // 2, c % 2
        in_maps.append({
            "xh": np.ascontiguousarray(xT[b][:, qh * QS:(qh + 1) * QS]),
            "ws": np.ascontiguousarray(w_cat[:, c * WS:(c + 1) * WS]),
            "bo": bo,
        })
    return run_bass_kernel_spmd(nc, in_maps, core_ids=list(range(NCORES)))


def _run_replicated(xT, w_cat, bo):
    if "nc_r" not in _CACHE:
        _CACHE["nc_r"] = _build_replicated()
    nc = _CACHE["nc_r"]
    # odd cores: token halves swapped so queries are always the first QS
    # columns (attention is permutation-invariant over keys; K and V permute
    # together)
    xT_sw = [np.ascontiguousarray(
        np.concatenate([t[:, QS:], t[:, :QS]], axis=1)) for t in xT]
    in_maps = []
    for c in range(NCORES):
        b, qh = c // 2, c % 2
        in_maps.append({
            "x": xT[b] if qh == 0 else xT_sw[b],
            "w": w_cat, "bo": bo,
        })
    return run_bass_kernel_spmd(nc, in_maps, core_ids=list(range(NCORES)))


def kernel(hidden_states, Wq, Wk, Wv, Wo, bo):
    global LAST_RESULTS
    xT, w_cat, bo = _prep_host(hidden_states, Wq, Wk, Wv, Wo, bo)

    if _CACHE.get("no_collectives"):
        res = _run_replicated(xT, w_cat, bo)
    else:
        try:
            res = _run_gather(xT, w_cat, bo)
        except Exception:
            _CACHE["no_collectives"] = True
            res = _run_replicated(xT, w_cat, bo)
    LAST_RESULTS = res

    out = np.empty((B, S, D), dtype=np.float32)
    for c in range(NCORES):
        b, qh = c // 2, c % 2
        out[b, qh * QS:(qh + 1) * QS, :] = res.results[c]["out"].astype(
            np.float32)
    return out



# revision 35
# speedup vs baseline: 2.1977x; 2.1977x over previous
"""Multi-head self-attention (CrossAttention with encoder_hidden_states=None)
on 8 Trainium2 NeuronCores.

Problem: hidden_states [B=4, S=2048, D=512], 8 heads x 64 dim, fp32 in/out.
    q/k/v = x @ W{q,k,v};  per-head softmax(q k^T / 8) v;  out proj + bias.

Sharding: core c = (batch b = c//2, query-half qh = c%2) handles a
1024-query slice of one batch element; K/V cover the full 2048 tokens.
Inputs are replicated per core (full x^T with the core's query half first,
full packed weights) so the kernel needs NO collectives; outputs are
disjoint slices -> pure concatenation.

Compute dataflow (bf16 operands, fp32 PSUM):
    QT[d, q]  = Wq^T xq^T            (4 tiles [128, 1024])
    KT[d, k]  = Wk^T x^T             (4 tiles [128, 2048])
    Vaug[k, h, 0:64] = x Wv slice, [.., 64] = 1   (16 tiles [128, 8, 65])
    S^T[k, q] = (KT_h)^T QT_h        (per head, 16 key-chunks)
    P^T       = exp(S^T / 8)         (ACT, unnormalized, bf16; uniform
                                      [128, 1536] batches double-buffered
                                      in 2x3 PSUM banks)
    O[q, 65]  = (P^T chunk)^T [V_h | 1]   (q on partitions; col 64
                                      accumulates the softmax denominator)
    A[q, 64h:64h+64] = O[:, 0:64] * 1/O[:, 64]    (DVE per-partition scalar)
    AT = A^T (DMA-engine xbar transpose), out = AT^T Wo + bo

The exp stream is the roofline for this shape; scores, trailing AV and
the projection matmuls are metered into the PE gaps between exp batches
by a budgeted background-work queue. Warm-up matmuls run during the
initial DMA loads so the PE p-state ramp is paid before real work.
"""

import numpy as np
import ml_dtypes

import concourse.bass as bass
import concourse.mybir as mybir
import concourse.tile as tile
from concourse import bacc
from concourse.bass_utils import run_bass_kernel_spmd
from contextlib import ExitStack

F32 = mybir.dt.float32
BF16 = mybir.dt.bfloat16

B, S, D = 4, 2048, 512
H, DH = 8, 64
SCALE = DH ** -0.5  # 0.125
NCORES = 8
QS = S // 2          # query tokens per core (1024)
KC = D // 128        # feature chunks (4)
TC = S // 128        # key token chunks (16)
QT8 = QS // 128      # query tiles (8)
WQ, WK, WV, WO = 0, D, 2 * D, 3 * D
BCOLS = 1536         # exp batch columns (3 PSUM banks)

_CACHE = {}
LAST_RESULTS = None


def _build():
    nc = bacc.Bacc("TRN2", target_bir_lowering=False, debug=False,
                   enable_asserts=False)

    x = nc.dram_tensor("x", [D, S], BF16, kind="ExternalInput").ap()
    w = nc.dram_tensor("w", [D, 4 * D], BF16, kind="ExternalInput").ap()
    bo = nc.dram_tensor("bo", [D], F32, kind="ExternalInput").ap()
    out = nc.dram_tensor("out", [QS, D], BF16, kind="ExternalOutput").ap()

    with tile.TileContext(nc) as tc, ExitStack() as ctx:
        xp = ctx.enter_context(tc.tile_pool(name="xp", bufs=4))
        wp = ctx.enter_context(tc.tile_pool(name="wp", bufs=1))
        ktp = ctx.enter_context(tc.tile_pool(name="ktp", bufs=4))
        qtp = ctx.enter_context(tc.tile_pool(name="qtp", bufs=4))
        vap = ctx.enter_context(tc.tile_pool(name="vap", bufs=16))
        ppp = ctx.enter_context(tc.tile_pool(name="ppp", bufs=3))
        ap_ = ctx.enter_context(tc.tile_pool(name="ap_", bufs=32))
        atp = ctx.enter_context(tc.tile_pool(name="atp", bufs=32))
        rcpp = ctx.enter_context(tc.tile_pool(name="rcpp", bufs=4))
        outp = ctx.enter_context(tc.tile_pool(name="outp", bufs=4))
        singles = ctx.enter_context(tc.tile_pool(name="singles", bufs=1))
        psSc = ctx.enter_context(tc.tile_pool(name="psSc", bufs=2,
                                              space="PSUM"))
        psA = ctx.enter_context(tc.tile_pool(name="psA", bufs=2,
                                             space="PSUM"))

        # ---- warm-up fodder (PE p-state ramp during DMA loads) ----------
        junk = singles.tile([128, 512], BF16)
        nc.vector.memset(junk, 0.0)

        # ---- DMA loads (HWDGE slots are the scarce resource: few + big)
        x_t = [xp.tile([128, S], BF16, tag="xp", name=f"x{kc}")
               for kc in range(KC)]
        w_big = wp.tile([128, KC, 4 * D], BF16, tag="wp", name="w_big")
        w_t = [w_big[:, kc, :] for kc in range(KC)]
        # dc0 stripes of Wq and Wk (all the first head-pair's projections
        # need), one combined DMA into a separate small tile.
        ws0 = wp.tile([128, KC, 2, 128], BF16, tag="ws0", name="ws0")
        w_dram = w.rearrange("(k p) s -> p k s", p=128)
        for i, off in enumerate((WQ, WK)):
            nc.sync.dma_start(out=ws0[:, :, i, :],
                              in_=w_dram[:, :, off:off + 128])
        for kc in range(KC):
            nc.sync.dma_start(out=x_t[kc][:, 0:QS],
                              in_=x[kc * 128:(kc + 1) * 128, 0:QS])
        for kc in range(KC):
            nc.sync.dma_start(out=x_t[kc][:, QS:S],
                              in_=x[kc * 128:(kc + 1) * 128, QS:S])
        nc.sync.dma_start(out=w_big[:, :, WQ:WK + D],
                          in_=w_dram[:, :, WQ:WK + D])
        nc.sync.dma_start(out=w_big[:, :, WV:WO + D],
                          in_=w_dram[:, :, WV:WO + D])

        bo_b = singles.tile([128, D], F32)
        bo_bcast = bass.AP(tensor=bo.tensor, offset=bo.offset,
                           ap=[[0, 128]] + list(bo.ap))
        nc.sync.dma_start(out=bo_b, in_=bo_bcast)

        # warm-up matmuls: keep PE busy from ~t=1us so the p-state ramp
        # (3us of continuous execution) completes while DMAs stream in.
        jps = psA.tile([128, 512], F32, tag="psA", name="jps")
        for _ in range(8):
            nc.tensor.matmul(jps, junk[:, 0:128], junk,
                             start=True, stop=True)

        # ---- persistent SBUF tensors ------------------------------------
        kt = [ktp.tile([128, S], BF16, tag="ktp", name=f"kt{dc}")
              for dc in range(KC)]
        qt = [qtp.tile([128, QS], BF16, tag="qtp", name=f"qt{dc}")
              for dc in range(KC)]
        va = [vap.tile([128, H, DH + 1], BF16, tag="vap", name=f"va{tci}")
              for tci in range(TC)]
        a_t = [[ap_.tile([128, 128], BF16, tag="ap_", name=f"a{p}_{qti}")
                for qti in range(QT8)] for p in range(KC)]
        at_t = [[atp.tile([128, 128], BF16, tag="atp", name=f"at{p}_{qti}")
                 for qti in range(QT8)] for p in range(KC)]
        pp = [ppp.tile([128, TC * QS], BF16, tag="ppp", name=f"pp{i}")
              for i in range(3)]

        ones_h = singles.tile([128, H, 1], BF16)
        nc.vector.memset(ones_h, 1.0)

        # ---- background work items --------------------------------------
        def _split_grp(lhs_of, rhs_of, copy_fn):
            """Two half-items: 2 accumulating matmuls each, sharing one
            PSUM tile; the second half emits the SBUF copy."""
            state = {}

            def part1():
                ps = psA.tile([128, 512], F32, tag="psA", name="psg")
                state["ps"] = ps
                for kc in (0, 1):
                    nc.tensor.matmul(ps, lhs_of(kc), rhs_of(kc),
                                     start=(kc == 0), stop=False)

            def part2():
                ps = state["ps"]
                for kc in (2, 3):
                    nc.tensor.matmul(ps, lhs_of(kc), rhs_of(kc),
                                     start=False, stop=(kc == KC - 1))
                copy_fn(ps)

            return part1, part2

        def _wq(dc, kc):
            if dc == 0:
                return ws0[:, kc, 0, :]
            return w_t[kc][:, WQ + dc * 128:WQ + (dc + 1) * 128]

        def _wk(dc, kc):
            if dc == 0:
                return ws0[:, kc, 1, :]
            return w_t[kc][:, WK + dc * 128:WK + (dc + 1) * 128]

        def _ps_copy(out, in_, eng=None):
            # PSUM -> SBUF evacuation; GPSIMD cannot touch PSUM on real HW.
            if eng == "scalar":
                nc.scalar.activation(
                    out=out, in_=in_,
                    func=mybir.ActivationFunctionType.Copy)
            else:
                nc.vector.tensor_copy(out=out, in_=in_)

        def qt_half(dc, j, copy_eng=None):
            return _split_grp(
                lambda kc: _wq(dc, kc),
                lambda kc: x_t[kc][:, j * 512:(j + 1) * 512],
                lambda ps: _ps_copy(
                    qt[dc][:, j * 512:(j + 1) * 512], ps, copy_eng))

        def kt_cc(dc, cc, copy_eng=None):
            return _split_grp(
                lambda kc: _wk(dc, kc),
                lambda kc: x_t[kc][:, cc * 512:(cc + 1) * 512],
                lambda ps: _ps_copy(
                    kt[dc][:, cc * 512:(cc + 1) * 512], ps, copy_eng))

        def v_tile(tci):
            def copy(ps):
                nc.vector.tensor_copy(
                    out=va[tci][:, :, 0:DH],
                    in_=ps.rearrange("p (h d) -> p h d", h=H))
                nc.gpsimd.tensor_copy(out=va[tci][:, :, DH:DH + 1],
                                      in_=ones_h)
            return _split_grp(
                lambda kc: x_t[kc][:, tci * 128:(tci + 1) * 128],
                lambda kc: w_t[kc][:, WV:WV + D],
                copy)

        def av_norm(h, qti, pool=None):
            """AV accumulation + normalize (+ pair transpose) for one
            q-tile of head h."""
            src = pp[h % 3]

            def emit():
                pl = pool or psA
                av = pl.tile([128, DH + 1], F32, tag=pl.name, name="psav")
                for c in range(TC):
                    nc.tensor.matmul(
                        av,
                        src[:, c * QS + qti * 128:c * QS + (qti + 1) * 128],
                        va[c][:, h, :],
                        start=(c == 0), stop=(c == TC - 1))
                rcp = rcpp.tile([128, 1], F32, tag="rcpp", name="rcp")
                nc.vector.reciprocal(rcp, av[:, DH:DH + 1])
                nc.vector.tensor_scalar_mul(
                    a_t[h // 2][qti][:, (h % 2) * DH:(h % 2) * DH + DH],
                    av[:, 0:DH], rcp)
                if h % 2 == 1:
                    nc.scalar.dma_start_transpose(out=at_t[h // 2][qti],
                                                  in_=a_t[h // 2][qti])
            return emit

        obig = singles.tile([128, QT8, D], BF16)

        def outproj(qti):
            def emit():
                pl = psA if qti % 2 == 0 else psSc
                po = pl.tile([128, D], F32, tag=pl.name, name="pso")
                for p in range(KC):
                    nc.tensor.matmul(
                        po, at_t[p][qti], w_t[p][:, WO:WO + D],
                        start=(p == 0), stop=(p == KC - 1))
                nc.vector.tensor_add(obig[:, qti, :], po, bo_b)
            return emit

        # (avail_pos, cost_ns, emit_fn, kind); drained in order, skipping
        # not-yet-available items. Emission order IS dependency order for
        # the Tile framework, so:
        #  - av items only pop once every v item has been emitted
        #  - kt/qt items for a head pair are force-drained before that
        #    pair's first score batch
        work = []
        emitted = {"v": 0}

        def enqueue(pos, cost, fn, kind=""):
            work.append([pos, cost, fn, kind])

        def _pop(i):
            it = work.pop(i)
            it[2]()
            if it[3] == "v":
                emitted["v"] += 1
            return it[1]

        def pump(pos, budget):
            # AV items (cheap, deadline-bound, free the pp ring) first.
            spent = 0
            for prio in ("av", ""):
                i = 0
                while i < len(work):
                    kind = work[i][3]
                    ok = work[i][0] <= pos and spent + work[i][1] <= budget
                    if prio == "av" and not kind.startswith("av"):
                        ok = False
                    if kind.startswith("av") and emitted["v"] < 2 * TC:
                        ok = False
                    if ok:
                        spent += _pop(i)
                    else:
                        i += 1

        def drain(keys):
            i = 0
            while i < len(work):
                if work[i][3] in keys:
                    _pop(i)
                else:
                    i += 1

        HLF = 427   # half projection group (2 matmuls)
        AVN = 460   # single AV+norm item

        def enq_grp(pos, parts, kind=""):
            enqueue(pos, HLF, parts[0], kind)
            enqueue(pos, HLF, parts[1], kind)

        enq_grp((0, 3), kt_cc(0, 2), "k0late")  # keys 1024+: needs xk
        enq_grp((0, 3), kt_cc(0, 3), "k0late")
        enq_grp((0, 7), qt_half(1, 0), "p0")    # needs full Wq load
        enq_grp((0, 7), qt_half(1, 1), "p0")
        enq_grp((0, 7), kt_cc(1, 0), "p0")
        enq_grp((0, 7), kt_cc(1, 1), "p0")
        enq_grp((0, 7), kt_cc(1, 2), "p0")
        enq_grp((0, 7), kt_cc(1, 3), "p0")
        for tci in range(TC):
            enq_grp((0, 9), v_tile(tci), "v")   # needs Wv (late load)
        enq_grp((1, 0), qt_half(2, 0), "p1")
        enq_grp((1, 0), qt_half(2, 1), "p1")
        for cc in range(4):
            enq_grp((1, 0), kt_cc(2, cc), "p1")
        enq_grp((2, 0), qt_half(3, 0), "p2")
        enq_grp((2, 0), qt_half(3, 1), "p2")
        for cc in range(4):
            enq_grp((2, 0), kt_cc(3, cc), "p2")

        # ---- prologue: QT[0] halves + first half of KT[0]; kt copies on
        # DVE so they run concurrently with the gpsimd qt copies.
        def run_grp(parts):
            parts[0]()
            parts[1]()

        run_grp(qt_half(0, 0))
        run_grp(kt_cc(0, 0, copy_eng="scalar"))
        run_grp(qt_half(0, 1))
        run_grp(kt_cc(0, 1, copy_eng="scalar"))

        # ---- main attention stream --------------------------------------
        # exp batches per head: 10 x 1536 cols + 1 x 1024; head 0 starts
        # with two smaller batches so the ACT stream begins sooner.
        std_sizes = [BCOLS] * 10 + [1024]
        h0_sizes = [512, 512, 1024] + [BCOLS] * 9 + [512]

        for h in range(H):
            dc, hh = h // 2, h % 2
            ppt = pp[h % 3]
            col0 = 0
            sizes = h0_sizes if h == 0 else std_sizes
            if h in (2, 4, 6):
                # this pair's KT/QT projections MUST be emitted before any
                # score matmul that reads them (emission order = dep order)
                drain({f"p{h // 2 - 1}"})
            if h >= 3:
                # AV of head h-3 must be emitted before exp(h) rewrites its
                # pp ring slot
                drain({f"av{h - 3}"})
            for bi, ncols in enumerate(sizes):
                if h == 0 and bi == 6:
                    drain({"k0late"})
                ps = psSc.tile([128, ncols], F32, tag="psSc", name="pss")
                for m in range(ncols // 512):
                    col = col0 + m * 512
                    c, joff = col // QS, col % QS
                    nc.tensor.matmul(
                        ps[:, m * 512:(m + 1) * 512],
                        kt[dc][hh * DH:hh * DH + DH, c * 128:(c + 1) * 128],
                        qt[dc][hh * DH:hh * DH + DH, joff:joff + 512],
                        start=True, stop=True)
                nc.scalar.activation(
                    out=ppt[:, col0:col0 + ncols], in_=ps,
                    func=mybir.ActivationFunctionType.Exp, scale=SCALE)
                col0 += ncols
                pump((h, bi), 940)
            if h < H - 1:
                for qti in range(QT8):
                    enqueue((h + 1, 0), AVN, av_norm(h, qti), f"av{h}")

        # ---- tail: drain queue, head-7 AV, out projection ---------------
        leftover = sum(1 for it in work)
        if leftover:
            import sys
            print(f"[kernel] {leftover} bg items left to tail",
                  file=sys.stderr)
        pump((H, 0), 10 ** 9)
        for qti in range(QT8):
            # alternate PSUM pools -> 4-deep AV pipelining in the tail
            av_norm(7, qti, pool=(psA if qti % 2 == 0 else psSc))()
        def store_half(half):
            # out rows (qti*128 + p) from obig[p, qti, d], 4 q-tiles at a
            # time: DRAM AP iterated (p, qti, d) to match the SBUF source.
            o_ap = bass.AP(
                tensor=out.tensor,
                offset=out.offset + half * (QT8 // 2) * 128 * D,
                ap=[[D, 128], [128 * D, QT8 // 2], [1, D]])
            nc.sync.dma_start(
                out=o_ap,
                in_=obig[:, half * (QT8 // 2):(half + 1) * (QT8 // 2), :])

        for qti in range(QT8):
            outproj(qti)()
            if qti == QT8 // 2 - 1:
                store_half(0)
        store_half(1)

    nc.compile()
    return nc


def _prep_host(hidden_states, Wq, Wk, Wv, Wo, bo):
    hidden_states = np.asarray(hidden_states, dtype=np.float32)
    w_cat = np.concatenate(
        [np.asarray(a, dtype=np.float32) for a in (Wq, Wk, Wv, Wo)],
        axis=1).astype(ml_dtypes.bfloat16)
    bo = np.asarray(bo, dtype=np.float32)
    xT = [np.ascontiguousarray(hidden_states[b].T).astype(ml_dtypes.bfloat16)
          for b in range(B)]
    return xT, w_cat, bo


def kernel(hidden_states, Wq, Wk, Wv, Wo, bo):
    global LAST_RESULTS
    xT, w_cat, bo = _prep_host(hidden_states, Wq, Wk, Wv, Wo, bo)

    if "nc" not in _CACHE:
        _CACHE["nc"] = _build()
    nc = _CACHE["nc"]

    # odd cores: token halves swapped so queries are always the first QS
    # columns (attention is permutation-invariant over keys; K and V
    # permute together)
    xT_sw = [np.ascontiguousarray(
        np.concatenate([t[:, QS:], t[:, :QS]], axis=1)) for t in xT]
    in_maps = []
    for c in range(NCORES):
        b, qh = c // 2, c % 2
        in_maps.append({
            "x": xT[b] if qh == 0 else xT_sw[b],
            "w": w_cat, "bo": bo,
        })
    res = run_bass_kernel_spmd(nc, in_maps, core_ids=list(range(NCORES)))
    LAST_RESULTS = res

    out = np.empty((B, S, D), dtype=np.float32)
    for c in range(NCORES):
        b, qh = c // 2, c % 2
        out[b, qh * QS:(qh + 1) * QS, :] = res.results[c]["out"].astype(
            np.float32)
    return out


# revision 56
# speedup vs baseline: 2.2397x; 1.0191x over previous
"""Multi-head self-attention (CrossAttention with encoder_hidden_states=None)
on 8 Trainium2 NeuronCores.

Problem: hidden_states [B=4, S=2048, D=512], 8 heads x 64 dim, fp32 in/out.
    q/k/v = x @ W{q,k,v};  per-head softmax(q k^T / 8) v;  out proj + bias.

Sharding: core c = (batch b = c//2, query-half qh = c%2) handles a
1024-query slice of one batch element; K/V cover the full 2048 tokens.
Inputs are replicated per core (full x^T with the core's query half first,
full packed weights) so the kernel needs NO collectives; outputs are
disjoint slices -> pure concatenation.

Compute dataflow (bf16 operands, fp32 PSUM):
    QT[d, q]  = Wq^T xq^T            (4 tiles [128, 1024])
    KT[d, k]  = Wk^T x^T             (4 tiles [128, 2048])
    Vaug[k, h, 0:64] = x Wv slice, [.., 64] = 1   (16 tiles [128, 8, 65])
    S^T[k, q] = (KT_h)^T QT_h        (per head, 16 key-chunks)
    P^T       = exp(S^T / 8)         (ACT, unnormalized, bf16; uniform
                                      [128, 1536] batches double-buffered
                                      in 2x3 PSUM banks)
    O[q, 65]  = (P^T chunk)^T [V_h | 1]   (q on partitions; col 64
                                      accumulates the softmax denominator)
    A[q, 64h:64h+64] = O[:, 0:64] * 1/O[:, 64]    (DVE per-partition scalar)
    AT = A^T (DMA-engine xbar transpose), out = AT^T Wo + bo

The exp stream is the roofline for this shape; scores, trailing AV and
the projection matmuls are metered into the PE gaps between exp batches
by a budgeted background-work queue. Warm-up matmuls run during the
initial DMA loads so the PE p-state ramp is paid before real work.
"""

import numpy as np
import ml_dtypes

import concourse.bass as bass
import concourse.mybir as mybir
import concourse.tile as tile
from concourse import bacc
from concourse.bass_utils import run_bass_kernel_spmd
from contextlib import ExitStack

F32 = mybir.dt.float32
BF16 = mybir.dt.bfloat16

B, S, D = 4, 2048, 512
H, DH = 8, 64
SCALE = DH ** -0.5  # 0.125
NCORES = 8
QS = S // 2          # query tokens per core (1024)
KC = D // 128        # feature chunks (4)
TC = S // 128        # key token chunks (16)
QT8 = QS // 128      # query tiles (8)
WQ, WK, WV, WO = 0, D, 2 * D, 3 * D
BCOLS = 1536         # exp batch columns (3 PSUM banks)

_CACHE = {}
LAST_RESULTS = None


def _build():
    nc = bacc.Bacc("TRN2", target_bir_lowering=False, debug=False,
                   enable_asserts=False)

    x = nc.dram_tensor("x", [D, S], BF16, kind="ExternalInput").ap()
    w = nc.dram_tensor("w", [D, 4 * D], BF16, kind="ExternalInput").ap()
    bo = nc.dram_tensor("bo", [D], F32, kind="ExternalInput").ap()
    out = nc.dram_tensor("out", [QS, D], BF16, kind="ExternalOutput").ap()

    with tile.TileContext(nc) as tc, ExitStack() as ctx:
        xp = ctx.enter_context(tc.tile_pool(name="xp", bufs=4))
        wp = ctx.enter_context(tc.tile_pool(name="wp", bufs=1))
        ktp = ctx.enter_context(tc.tile_pool(name="ktp", bufs=4))
        qtp = ctx.enter_context(tc.tile_pool(name="qtp", bufs=4))
        vap = ctx.enter_context(tc.tile_pool(name="vap", bufs=16))
        ppp = ctx.enter_context(tc.tile_pool(name="ppp", bufs=3))
        ap_ = ctx.enter_context(tc.tile_pool(name="ap_", bufs=32))
        atp = ctx.enter_context(tc.tile_pool(name="atp", bufs=32))
        rcpp = ctx.enter_context(tc.tile_pool(name="rcpp", bufs=4))
        outp = ctx.enter_context(tc.tile_pool(name="outp", bufs=4))
        singles = ctx.enter_context(tc.tile_pool(name="singles", bufs=1))
        psSc = ctx.enter_context(tc.tile_pool(name="psSc", bufs=2,
                                              space="PSUM"))
        psA = ctx.enter_context(tc.tile_pool(name="psA", bufs=2,
                                             space="PSUM"))

        # ---- warm-up fodder (PE p-state ramp during DMA loads) ----------
        junk = singles.tile([128, 512], BF16)
        nc.vector.memset(junk, 0.0)

        # ---- DMA loads (HWDGE slots are the scarce resource: few + big)
        x_t = [xp.tile([128, S], BF16, tag="xp", name=f"x{kc}")
               for kc in range(KC)]
        w_big = wp.tile([128, KC, 4 * D], BF16, tag="wp", name="w_big")
        w_t = [w_big[:, kc, :] for kc in range(KC)]
        # dc0 stripes of Wq and Wk (all the first head-pair's projections
        # need), one combined DMA into a separate small tile.
        ws0 = wp.tile([128, KC, 2, 128], BF16, tag="ws0", name="ws0")
        w_dram = w.rearrange("(k p) s -> p k s", p=128)
        for i, off in enumerate((WQ, WK)):
            nc.sync.dma_start(out=ws0[:, :, i, :],
                              in_=w_dram[:, :, off:off + 128])
        for kc in range(KC):
            nc.sync.dma_start(out=x_t[kc][:, 0:QS],
                              in_=x[kc * 128:(kc + 1) * 128, 0:QS])
        for kc in range(KC):
            nc.sync.dma_start(out=x_t[kc][:, QS:S],
                              in_=x[kc * 128:(kc + 1) * 128, QS:S])
        nc.sync.dma_start(out=w_big[:, :, WQ:WK + D],
                          in_=w_dram[:, :, WQ:WK + D])
        nc.sync.dma_start(out=w_big[:, :, WV:WO + D],
                          in_=w_dram[:, :, WV:WO + D])

        bo_b = singles.tile([128, D], F32)
        bo_bcast = bass.AP(tensor=bo.tensor, offset=bo.offset,
                           ap=[[0, 128]] + list(bo.ap))
        nc.sync.dma_start(out=bo_b, in_=bo_bcast)

        # warm-up matmuls: keep PE busy from ~t=1us so the p-state ramp
        # (3us of continuous execution) completes while DMAs stream in.
        jps = psA.tile([128, 512], F32, tag="psA", name="jps")
        for _ in range(8):
            nc.tensor.matmul(jps, junk[:, 0:128], junk,
                             start=True, stop=True)

        # ---- persistent SBUF tensors ------------------------------------
        kt = [ktp.tile([128, S], BF16, tag="ktp", name=f"kt{dc}")
              for dc in range(KC)]
        qt = [qtp.tile([128, QS], BF16, tag="qtp", name=f"qt{dc}")
              for dc in range(KC)]
        va = [vap.tile([128, H, DH + 1], BF16, tag="vap", name=f"va{tci}")
              for tci in range(TC)]
        a_t = [[ap_.tile([128, 128], BF16, tag="ap_", name=f"a{p}_{qti}")
                for qti in range(QT8)] for p in range(KC)]
        at_t = [[atp.tile([128, 128], BF16, tag="atp", name=f"at{p}_{qti}")
                 for qti in range(QT8)] for p in range(KC)]
        pp = [ppp.tile([128, TC * QS], BF16, tag="ppp", name=f"pp{i}")
              for i in range(3)]

        ones_h = singles.tile([128, H, 1], BF16)
        nc.vector.memset(ones_h, 1.0)
        # identity matrix for PE-mode transposes: iota(c - p) == 0
        idt_i = singles.tile([128, 128], mybir.dt.int32)
        nc.gpsimd.iota(idt_i, [[1, 128]], channel_multiplier=-1)
        id_t = singles.tile([128, 128], BF16)
        nc.vector.tensor_scalar(id_t, idt_i, 0, None,
                                op0=mybir.AluOpType.is_equal)

        # ---- background work items --------------------------------------
        def _split_grp(lhs_of, rhs_of, copy_fn):
            """Two half-items: 2 accumulating matmuls each, sharing one
            PSUM tile; the second half emits the SBUF copy."""
            state = {}

            def part1():
                ps = psA.tile([128, 512], F32, tag="psA", name="psg")
                state["ps"] = ps
                for kc in (0, 1):
                    nc.tensor.matmul(ps, lhs_of(kc), rhs_of(kc),
                                     start=(kc == 0), stop=False)

            def part2():
                ps = state["ps"]
                for kc in (2, 3):
                    nc.tensor.matmul(ps, lhs_of(kc), rhs_of(kc),
                                     start=False, stop=(kc == KC - 1))
                copy_fn(ps)

            return part1, part2

        def _wq(dc, kc):
            if dc == 0:
                return ws0[:, kc, 0, :]
            return w_t[kc][:, WQ + dc * 128:WQ + (dc + 1) * 128]

        def _wk(dc, kc):
            if dc == 0:
                return ws0[:, kc, 1, :]
            return w_t[kc][:, WK + dc * 128:WK + (dc + 1) * 128]

        def _ps_copy(out, in_, eng=None):
            # PSUM -> SBUF evacuation; GPSIMD cannot touch PSUM on real HW.
            if eng == "scalar":
                nc.scalar.activation(
                    out=out, in_=in_,
                    func=mybir.ActivationFunctionType.Copy)
            else:
                nc.vector.tensor_copy(out=out, in_=in_)

        def qt_half(dc, j, copy_eng=None):
            return _split_grp(
                lambda kc: _wq(dc, kc),
                lambda kc: x_t[kc][:, j * 512:(j + 1) * 512],
                lambda ps: _ps_copy(
                    qt[dc][:, j * 512:(j + 1) * 512], ps, copy_eng))

        def kt_cc(dc, cc, copy_eng=None):
            return _split_grp(
                lambda kc: _wk(dc, kc),
                lambda kc: x_t[kc][:, cc * 512:(cc + 1) * 512],
                lambda ps: _ps_copy(
                    kt[dc][:, cc * 512:(cc + 1) * 512], ps, copy_eng))

        def v_tile(tci):
            def copy(ps):
                nc.vector.tensor_copy(
                    out=va[tci][:, :, 0:DH],
                    in_=ps.rearrange("p (h d) -> p h d", h=H))
                nc.gpsimd.tensor_copy(out=va[tci][:, :, DH:DH + 1],
                                      in_=ones_h)
            return _split_grp(
                lambda kc: x_t[kc][:, tci * 128:(tci + 1) * 128],
                lambda kc: w_t[kc][:, WV:WV + D],
                copy)

        def av_norm(h, qti, pool=None):
            """AV accumulation + normalize (+ pair transpose) for one
            q-tile of head h."""
            src = pp[h % 3]

            def emit():
                pl = pool or psA
                av = pl.tile([128, DH + 1], F32, tag=pl.name, name="psav")

                for c in range(TC):
                    nc.tensor.matmul(
                        av,
                        src[:, c * QS + qti * 128:c * QS + (qti + 1) * 128],
                        va[c][:, h, :],
                        start=(c == 0), stop=(c == TC - 1))
                rcp = rcpp.tile([128, 1], F32, tag="rcpp", name="rcp")
                nc.vector.reciprocal(rcp, av[:, DH:DH + 1])
                nc.vector.tensor_scalar_mul(
                    a_t[h // 2][qti][:, (h % 2) * DH:(h % 2) * DH + DH],
                    av[:, 0:DH], rcp)
                if h == 7:
                    pst = pl.tile([128, 128], BF16, tag=pl.name,
                                  name="pst")
                    nc.tensor.transpose(pst, a_t[3][qti], id_t)
                    nc.scalar.activation(
                        out=at_t[3][qti], in_=pst,
                        func=mybir.ActivationFunctionType.Copy)
                elif h % 2 == 1:
                    nc.scalar.dma_start_transpose(out=at_t[h // 2][qti],
                                                  in_=a_t[h // 2][qti])
            return emit

        obig = singles.tile([128, QT8, D], BF16)
        oacc = singles.tile([128, QT8, D], BF16)

        def op_part(qti, p):
            def emit():
                po = psA.tile([128, D], F32, tag="psA", name="pop")
                nc.tensor.matmul(po, at_t[p][qti], w_t[p][:, WO:WO + D],
                                 start=True, stop=True)
                if p == 0:
                    nc.vector.tensor_add(oacc[:, qti, :], po, bo_b)
                else:
                    nc.vector.tensor_add(oacc[:, qti, :], oacc[:, qti, :],
                                         po)
            return emit

        def outproj_final(qti):
            def emit():
                pl = psA if qti % 2 == 0 else psSc
                po = pl.tile([128, D], F32, tag=pl.name, name="pso")
                nc.tensor.matmul(po, at_t[KC - 1][qti],
                                 w_t[KC - 1][:, WO:WO + D],
                                 start=True, stop=True)
                nc.vector.tensor_add(obig[:, qti, :], oacc[:, qti, :], po)
            return emit

        # (avail_pos, cost_ns, emit_fn, kind); drained in order, skipping
        # not-yet-available items. Emission order IS dependency order for
        # the Tile framework, so:
        #  - av items only pop once every v item has been emitted
        #  - kt/qt items for a head pair are force-drained before that
        #    pair's first score batch
        work = []
        emitted = {"v": 0}
        op_gate = {}

        def enqueue(pos, cost, fn, kind=""):
            work.append([pos, cost, fn, kind])

        def _pop(i):
            it = work.pop(i)
            it[2]()
            emitted[it[3]] = emitted.get(it[3], 0) + 1
            return it[1]

        def pump(pos, budget):
            # AV items (cheap, deadline-bound, free the pp ring) first.
            spent = 0
            for prio in ("av", ""):
                i = 0
                while i < len(work):
                    kind = work[i][3]
                    ok = work[i][0] <= pos and spent + work[i][1] <= budget
                    if prio == "av" and not kind.startswith("av"):
                        ok = False
                    if kind.startswith("av") and emitted["v"] < 2 * TC:
                        ok = False
                    if kind in op_gate and emitted.get(op_gate[kind], 0) < QT8:
                        ok = False
                    if ok:
                        spent += _pop(i)
                    else:
                        i += 1

        def drain(keys):
            i = 0
            while i < len(work):
                if work[i][3] in keys:
                    _pop(i)
                else:
                    i += 1

        HLF = 427   # half projection group (2 matmuls)
        AVN = 460   # single AV+norm item

        def enq_grp(pos, parts, kind=""):
            enqueue(pos, HLF, parts[0], kind)
            enqueue(pos, HLF, parts[1], kind)

        enq_grp((0, 3), kt_cc(0, 2), "k0late")  # keys 1024+: needs xk
        enq_grp((0, 3), kt_cc(0, 3), "k0late")
        enq_grp((0, 7), qt_half(1, 0), "p0")    # needs full Wq load
        enq_grp((0, 7), qt_half(1, 1), "p0")
        enq_grp((0, 7), kt_cc(1, 0), "p0")
        enq_grp((0, 7), kt_cc(1, 1), "p0")
        enq_grp((0, 7), kt_cc(1, 2), "p0")
        enq_grp((0, 7), kt_cc(1, 3), "p0")
        for tci in range(TC):
            enq_grp((0, 9), v_tile(tci), "v")   # needs Wv (late load)
        enq_grp((1, 0), qt_half(2, 0), "p1")
        enq_grp((1, 0), qt_half(2, 1), "p1")
        for cc in range(4):
            enq_grp((1, 0), kt_cc(2, cc), "p1")
        enq_grp((2, 0), qt_half(3, 0), "p2")
        enq_grp((2, 0), qt_half(3, 1), "p2")
        for cc in range(4):
            enq_grp((2, 0), kt_cc(3, cc), "p2")

        # ---- prologue: QT[0] halves + first half of KT[0]; kt copies on
        # DVE so they run concurrently with the gpsimd qt copies.
        def run_grp(parts):
            parts[0]()
            parts[1]()

        run_grp(qt_half(0, 0))
        run_grp(kt_cc(0, 0, copy_eng="scalar"))
        run_grp(qt_half(0, 1))
        enq_grp((0, 1), kt_cc(0, 1), "k01")

        OPP = 250   # partial out-projection item (1 matmul + DVE add)
        for p in range(KC - 1):
            op_gate[f"op{p}"] = f"av{2 * p + 1}"
            for qti in range(QT8):
                enqueue((2 * p + 2, 0), OPP, op_part(qti, p), f"op{p}")

        # ---- main attention stream --------------------------------------
        # exp batches per head: 10 x 1536 cols + 1 x 1024; head 0 starts
        # with two smaller batches so the ACT stream begins sooner.
        std_sizes = [BCOLS] * 10 + [1024]
        h0_sizes = [512, 512, 1024] + [BCOLS] * 9 + [512]

        for h in range(H):
            dc, hh = h // 2, h % 2
            ppt = pp[h % 3]
            col0 = 0
            sizes = h0_sizes if h == 0 else std_sizes
            if h in (2, 4, 6):
                # this pair's KT/QT projections MUST be emitted before any
                # score matmul that reads them (emission order = dep order)
                drain({f"p{h // 2 - 1}"})
            if h >= 3:
                # AV of head h-3 must be emitted before exp(h) rewrites its
                # pp ring slot (and the V tiles it reads before that)
                if emitted["v"] < 2 * TC:
                    drain({"v"})
                drain({f"av{h - 3}"})
            for bi, ncols in enumerate(sizes):
                if h == 0 and bi == 2:
                    drain({"k01"})
                if h == 0 and bi == 6:
                    drain({"k0late"})
                ps = psSc.tile([128, ncols], F32, tag="psSc", name="pss")
                for m in range(ncols // 512):
                    col = col0 + m * 512
                    c, joff = col // QS, col % QS
                    nc.tensor.matmul(
                        ps[:, m * 512:(m + 1) * 512],
                        kt[dc][hh * DH:hh * DH + DH, c * 128:(c + 1) * 128],
                        qt[dc][hh * DH:hh * DH + DH, joff:joff + 512],
                        start=True, stop=True)
                nc.scalar.activation(
                    out=ppt[:, col0:col0 + ncols], in_=ps,
                    func=mybir.ActivationFunctionType.Exp, scale=SCALE)
                col0 += ncols
                pump((h, bi), 900)
            if h < H - 1:
                for qti in range(QT8):
                    enqueue((h + 1, 0), AVN, av_norm(h, qti), f"av{h}")

        # ---- tail: drain queue, head-7 AV, out projection ---------------
        leftover = sum(1 for it in work)
        if leftover:
            import sys
            print(f"[kernel] {leftover} bg items left to tail",
                  file=sys.stderr)
        pump((H, 0), 10 ** 9)
        for qti in range(QT8):
            # alternate PSUM pools -> 4-deep AV pipelining in the tail;
            # final out-projections trail 4 q-tiles behind so their psum
            # inputs (norm -> PE transpose -> ACT copy) are surely ready.
            av_norm(7, qti, pool=(psA if qti % 2 == 0 else psSc))()
            if qti >= 4:
                outproj_final(qti - 4)()

        def store_half(half):
            # out rows (qti*128 + p) from obig[p, qti, d], 4 q-tiles at a
            # time: DRAM AP iterated (p, qti, d) to match the SBUF source.
            o_ap = bass.AP(
                tensor=out.tensor,
                offset=out.offset + half * (QT8 // 2) * 128 * D,
                ap=[[D, 128], [128 * D, QT8 // 2], [1, D]])
            nc.sync.dma_start(
                out=o_ap,
                in_=obig[:, half * (QT8 // 2):(half + 1) * (QT8 // 2), :])

        store_half(0)
        for qti in range(QT8 - 4, QT8):
            outproj_final(qti)()
        store_half(1)

    nc.compile()
    return nc


def _prep_host(hidden_states, Wq, Wk, Wv, Wo, bo):
    hidden_states = np.asarray(hidden_states, dtype=np.float32)
    w_cat = np.concatenate(
        [np.asarray(a, dtype=np.float32) for a in (Wq, Wk, Wv, Wo)],
        axis=1).astype(ml_dtypes.bfloat16)
    bo = np.asarray(bo, dtype=np.float32)
    xT = [np.ascontiguousarray(hidden_states[b].T).astype(ml_dtypes.bfloat16)
          for b in range(B)]
    return xT, w_cat, bo


def kernel(hidden_states, Wq, Wk, Wv, Wo, bo):
    global LAST_RESULTS
    xT, w_cat, bo = _prep_host(hidden_states, Wq, Wk, Wv, Wo, bo)

    if "nc" not in _CACHE:
        _CACHE["nc"] = _build()
    nc = _CACHE["nc"]

    # odd cores: token halves swapped so queries are always the first QS
    # columns (attention is permutation-invariant over keys; K and V
    # permute together)
    xT_sw = [np.ascontiguousarray(
        np.concatenate([t[:, QS:], t[:, :QS]], axis=1)) for t in xT]
    in_maps = []
    for c in range(NCORES):
        b, qh = c // 2, c % 2
        in_maps.append({
            "x": xT[b] if qh == 0 else xT_sw[b],
            "w": w_cat, "bo": bo,
        })
    res = run_bass_kernel_spmd(nc, in_maps, core_ids=list(range(NCORES)))
    LAST_RESULTS = res

    out = np.empty((B, S, D), dtype=np.float32)
    for c in range(NCORES):
        b, qh = c // 2, c % 2
        out[b, qh * QS:(qh + 1) * QS, :] = res.results[c]["out"].astype(
            np.float32)
    return out


# revision 60
# speedup vs baseline: 2.2427x; 1.0013x over previous
"""Multi-head self-attention (CrossAttention with encoder_hidden_states=None)
on 8 Trainium2 NeuronCores.

Problem: hidden_states [B=4, S=2048, D=512], 8 heads x 64 dim, fp32 in/out.
    q/k/v = x @ W{q,k,v};  per-head softmax(q k^T / 8) v;  out proj + bias.

Sharding: core c = (batch b = c//2, query-half qh = c%2) handles a
1024-query slice of one batch element; K/V cover the full 2048 tokens.
Inputs are replicated per core (full x^T with the core's query half first,
full packed weights) so the kernel needs NO collectives; outputs are
disjoint slices -> pure concatenation.

Compute dataflow (bf16 operands, fp32 PSUM):
    QT[d, q]  = Wq^T xq^T            (4 tiles [128, 1024])
    KT[d, k]  = Wk^T x^T             (4 tiles [128, 2048])
    Vaug[k, h, 0:64] = x Wv slice, [.., 64] = 1   (16 tiles [128, 8, 65])
    S^T[k, q] = (KT_h)^T QT_h        (per head, 16 key-chunks)
    P^T       = exp(S^T / 8)         (ACT, unnormalized, bf16; uniform
                                      [128, 1536] batches double-buffered
                                      in 2x3 PSUM banks)
    O[q, 65]  = (P^T chunk)^T [V_h | 1]   (q on partitions; col 64
                                      accumulates the softmax denominator)
    A[q, 64h:64h+64] = O[:, 0:64] * 1/O[:, 64]    (DVE per-partition scalar)
    AT = A^T (DMA-engine xbar transpose), out = AT^T Wo + bo

The exp stream is the roofline for this shape; scores, trailing AV and
the projection matmuls are metered into the PE gaps between exp batches
by a budgeted background-work queue. Warm-up matmuls run during the
initial DMA loads so the PE p-state ramp is paid before real work.
"""

import numpy as np
import ml_dtypes

import concourse.bass as bass
import concourse.mybir as mybir
import concourse.tile as tile
from concourse import bacc
from concourse.bass_utils import run_bass_kernel_spmd
from contextlib import ExitStack

F32 = mybir.dt.float32
BF16 = mybir.dt.bfloat16

B, S, D = 4, 2048, 512
H, DH = 8, 64
SCALE = DH ** -0.5  # 0.125
NCORES = 8
QS = S // 2          # query tokens per core (1024)
KC = D // 128        # feature chunks (4)
TC = S // 128        # key token chunks (16)
QT8 = QS // 128      # query tiles (8)
WQ, WK, WV, WO = 0, D, 2 * D, 3 * D
BCOLS = 1536         # exp batch columns (3 PSUM banks)

_CACHE = {}
LAST_RESULTS = None


def _build():
    nc = bacc.Bacc("TRN2", target_bir_lowering=False, debug=False,
                   enable_asserts=False)

    x = nc.dram_tensor("x", [D, S], BF16, kind="ExternalInput").ap()
    w = nc.dram_tensor("w", [D, 4 * D], BF16, kind="ExternalInput").ap()
    bo = nc.dram_tensor("bo", [D], F32, kind="ExternalInput").ap()
    out = nc.dram_tensor("out", [QS, D], BF16, kind="ExternalOutput").ap()

    with tile.TileContext(nc) as tc, ExitStack() as ctx:
        xp = ctx.enter_context(tc.tile_pool(name="xp", bufs=4))
        wp = ctx.enter_context(tc.tile_pool(name="wp", bufs=1))
        ktp = ctx.enter_context(tc.tile_pool(name="ktp", bufs=4))
        qtp = ctx.enter_context(tc.tile_pool(name="qtp", bufs=4))
        vap = ctx.enter_context(tc.tile_pool(name="vap", bufs=16))
        ppp = ctx.enter_context(tc.tile_pool(name="ppp", bufs=3))
        ap_ = ctx.enter_context(tc.tile_pool(name="ap_", bufs=32))
        atp = ctx.enter_context(tc.tile_pool(name="atp", bufs=32))
        rcpp = ctx.enter_context(tc.tile_pool(name="rcpp", bufs=4))
        outp = ctx.enter_context(tc.tile_pool(name="outp", bufs=4))
        singles = ctx.enter_context(tc.tile_pool(name="singles", bufs=1))
        psSc = ctx.enter_context(tc.tile_pool(name="psSc", bufs=2,
                                              space="PSUM"))
        psA = ctx.enter_context(tc.tile_pool(name="psA", bufs=2,
                                             space="PSUM"))

        # ---- warm-up fodder (PE p-state ramp during DMA loads) ----------
        junk = singles.tile([128, 512], BF16)
        nc.vector.memset(junk, 0.0)

        # ---- DMA loads (HWDGE slots are the scarce resource: few + big)
        x_t = [xp.tile([128, S], BF16, tag="xp", name=f"x{kc}")
               for kc in range(KC)]
        w_big = wp.tile([128, KC, 4 * D], BF16, tag="wp", name="w_big")
        w_t = [w_big[:, kc, :] for kc in range(KC)]
        # dc0 stripes of Wq and Wk (all the first head-pair's projections
        # need), one combined DMA into a separate small tile.
        ws0 = wp.tile([128, KC, 2, 128], BF16, tag="ws0", name="ws0")
        w_dram = w.rearrange("(k p) s -> p k s", p=128)
        for i, off in enumerate((WQ, WK)):
            nc.sync.dma_start(out=ws0[:, :, i, :],
                              in_=w_dram[:, :, off:off + 128])
        for kc in range(KC):
            nc.sync.dma_start(out=x_t[kc][:, 0:QS],
                              in_=x[kc * 128:(kc + 1) * 128, 0:QS])
        for kc in range(KC):
            nc.sync.dma_start(out=x_t[kc][:, QS:S],
                              in_=x[kc * 128:(kc + 1) * 128, QS:S])
        nc.sync.dma_start(out=w_big[:, :, WQ:WK + D],
                          in_=w_dram[:, :, WQ:WK + D])
        nc.sync.dma_start(out=w_big[:, :, WV:WO + D],
                          in_=w_dram[:, :, WV:WO + D])

        bo_b = singles.tile([128, D], F32)
        bo_bcast = bass.AP(tensor=bo.tensor, offset=bo.offset,
                           ap=[[0, 128]] + list(bo.ap))
        nc.sync.dma_start(out=bo_b, in_=bo_bcast)

        # warm-up matmuls: keep PE busy from ~t=1us so the p-state ramp
        # (3us of continuous execution) completes while DMAs stream in.
        jps = psA.tile([128, 512], F32, tag="psA", name="jps")
        for _ in range(8):
            nc.tensor.matmul(jps, junk[:, 0:128], junk,
                             start=True, stop=True)

        # ---- persistent SBUF tensors ------------------------------------
        kt = [ktp.tile([128, S], BF16, tag="ktp", name=f"kt{dc}")
              for dc in range(KC)]
        qt = [qtp.tile([128, QS], BF16, tag="qtp", name=f"qt{dc}")
              for dc in range(KC)]
        va = [vap.tile([128, H, DH + 1], BF16, tag="vap", name=f"va{tci}")
              for tci in range(TC)]
        a_t = [[ap_.tile([128, 128], BF16, tag="ap_", name=f"a{p}_{qti}")
                for qti in range(QT8)] for p in range(KC)]
        at_t = [[atp.tile([128, 128], BF16, tag="atp", name=f"at{p}_{qti}")
                 for qti in range(QT8)] for p in range(KC)]
        pp = [ppp.tile([128, TC * QS], BF16, tag="ppp", name=f"pp{i}")
              for i in range(3)]

        ones_h = singles.tile([128, H, 1], BF16)
        nc.vector.memset(ones_h, 1.0)
        # identity matrix for PE-mode transposes: iota(c - p) == 0
        idt_i = singles.tile([128, 128], mybir.dt.int32)
        nc.gpsimd.iota(idt_i, [[1, 128]], channel_multiplier=-1)
        id_t = singles.tile([128, 128], BF16)
        nc.vector.tensor_scalar(id_t, idt_i, 0, None,
                                op0=mybir.AluOpType.is_equal)

        # ---- background work items --------------------------------------
        def _split_grp(lhs_of, rhs_of, copy_fn):
            """Two half-items: 2 accumulating matmuls each, sharing one
            PSUM tile; the second half emits the SBUF copy."""
            state = {}

            def part1():
                ps = psA.tile([128, 512], F32, tag="psA", name="psg")
                state["ps"] = ps
                for kc in (0, 1):
                    nc.tensor.matmul(ps, lhs_of(kc), rhs_of(kc),
                                     start=(kc == 0), stop=False)

            def part2():
                ps = state["ps"]
                for kc in (2, 3):
                    nc.tensor.matmul(ps, lhs_of(kc), rhs_of(kc),
                                     start=False, stop=(kc == KC - 1))
                copy_fn(ps)

            return part1, part2

        def _wq(dc, kc):
            if dc == 0:
                return ws0[:, kc, 0, :]
            return w_t[kc][:, WQ + dc * 128:WQ + (dc + 1) * 128]

        def _wk(dc, kc):
            if dc == 0:
                return ws0[:, kc, 1, :]
            return w_t[kc][:, WK + dc * 128:WK + (dc + 1) * 128]

        def _ps_copy(out, in_, eng=None):
            # PSUM -> SBUF evacuation; GPSIMD cannot touch PSUM on real HW.
            if eng == "scalar":
                nc.scalar.activation(
                    out=out, in_=in_,
                    func=mybir.ActivationFunctionType.Copy)
            else:
                nc.vector.tensor_copy(out=out, in_=in_)

        def qt_half(dc, j, copy_eng=None):
            return _split_grp(
                lambda kc: _wq(dc, kc),
                lambda kc: x_t[kc][:, j * 512:(j + 1) * 512],
                lambda ps: _ps_copy(
                    qt[dc][:, j * 512:(j + 1) * 512], ps, copy_eng))

        def kt_cc(dc, cc, copy_eng=None):
            return _split_grp(
                lambda kc: _wk(dc, kc),
                lambda kc: x_t[kc][:, cc * 512:(cc + 1) * 512],
                lambda ps: _ps_copy(
                    kt[dc][:, cc * 512:(cc + 1) * 512], ps, copy_eng))

        def v_tile(tci):
            def copy(ps):
                nc.vector.tensor_copy(
                    out=va[tci][:, :, 0:DH],
                    in_=ps.rearrange("p (h d) -> p h d", h=H))
                nc.gpsimd.tensor_copy(out=va[tci][:, :, DH:DH + 1],
                                      in_=ones_h)
            return _split_grp(
                lambda kc: x_t[kc][:, tci * 128:(tci + 1) * 128],
                lambda kc: w_t[kc][:, WV:WV + D],
                copy)

        def av_norm(h, qti, pool=None):
            """AV accumulation + normalize (+ pair transpose) for one
            q-tile of head h."""
            src = pp[h % 3]

            def emit():
                pl = pool or psA
                av = pl.tile([128, DH + 1], F32, tag=pl.name, name="psav")

                for c in range(TC):
                    nc.tensor.matmul(
                        av,
                        src[:, c * QS + qti * 128:c * QS + (qti + 1) * 128],
                        va[c][:, h, :],
                        start=(c == 0), stop=(c == TC - 1))
                rcp = rcpp.tile([128, 1], F32, tag="rcpp", name="rcp")
                nc.vector.reciprocal(rcp, av[:, DH:DH + 1])
                nc.vector.tensor_scalar_mul(
                    a_t[h // 2][qti][:, (h % 2) * DH:(h % 2) * DH + DH],
                    av[:, 0:DH], rcp)
                if h == 7:
                    pst = pl.tile([128, 128], BF16, tag=pl.name,
                                  name="pst")
                    nc.tensor.transpose(pst, a_t[3][qti], id_t)
                    nc.scalar.activation(
                        out=at_t[3][qti], in_=pst,
                        func=mybir.ActivationFunctionType.Copy)
                elif h % 2 == 1:
                    nc.scalar.dma_start_transpose(out=at_t[h // 2][qti],
                                                  in_=a_t[h // 2][qti])
            return emit

        obig = singles.tile([128, QT8, D], BF16)
        oacc = singles.tile([128, QT8, D], BF16)

        def op_part(qti, p):
            def emit():
                po = psA.tile([128, D], F32, tag="psA", name="pop")
                nc.tensor.matmul(po, at_t[p][qti], w_t[p][:, WO:WO + D],
                                 start=True, stop=True)
                if p == 0:
                    nc.vector.tensor_add(oacc[:, qti, :], po, bo_b)
                else:
                    nc.vector.tensor_add(oacc[:, qti, :], oacc[:, qti, :],
                                         po)
            return emit

        def outproj_final(qti):
            def emit():
                pl = psA if qti % 2 == 0 else psSc
                po = pl.tile([128, D], F32, tag=pl.name, name="pso")
                nc.tensor.matmul(po, at_t[KC - 1][qti],
                                 w_t[KC - 1][:, WO:WO + D],
                                 start=True, stop=True)
                nc.vector.tensor_add(obig[:, qti, :], oacc[:, qti, :], po)
            return emit

        # (avail_pos, cost_ns, emit_fn, kind); drained in order, skipping
        # not-yet-available items. Emission order IS dependency order for
        # the Tile framework, so:
        #  - av items only pop once every v item has been emitted
        #  - kt/qt items for a head pair are force-drained before that
        #    pair's first score batch
        work = []
        emitted = {"v": 0}
        op_gate = {}

        def enqueue(pos, cost, fn, kind=""):
            work.append([pos, cost, fn, kind])

        def _pop(i):
            it = work.pop(i)
            it[2]()
            emitted[it[3]] = emitted.get(it[3], 0) + 1
            return it[1]

        def pump(pos, budget):
            # AV items (cheap, deadline-bound, free the pp ring) first.
            spent = 0
            for prio in ("av", ""):
                i = 0
                while i < len(work):
                    kind = work[i][3]
                    ok = work[i][0] <= pos and spent + work[i][1] <= budget
                    if prio == "av" and not kind.startswith("av"):
                        ok = False
                    if kind.startswith("av") and emitted["v"] < 2 * TC:
                        ok = False
                    if kind in op_gate and emitted.get(op_gate[kind], 0) < QT8:
                        ok = False
                    if ok:
                        spent += _pop(i)
                    else:
                        i += 1

        def drain(keys):
            i = 0
            while i < len(work):
                if work[i][3] in keys:
                    _pop(i)
                else:
                    i += 1

        HLF = 427   # half projection group (2 matmuls)
        AVN = 460   # single AV+norm item

        def enq_grp(pos, parts, kind=""):
            enqueue(pos, HLF, parts[0], kind)
            enqueue(pos, HLF, parts[1], kind)

        enq_grp((0, 3), kt_cc(0, 2), "k0late")  # keys 1024+: needs xk
        enq_grp((0, 3), kt_cc(0, 3), "k0late")
        enq_grp((0, 7), qt_half(1, 0), "p0")    # needs full Wq load
        enq_grp((0, 7), qt_half(1, 1), "p0")
        enq_grp((0, 7), kt_cc(1, 0), "p0")
        enq_grp((0, 7), kt_cc(1, 1), "p0")
        enq_grp((0, 7), kt_cc(1, 2), "p0")
        enq_grp((0, 7), kt_cc(1, 3), "p0")
        for tci in range(TC):
            enq_grp((0, 9), v_tile(tci), "v")   # needs Wv (late load)
        enq_grp((1, 0), qt_half(2, 0), "p1")
        enq_grp((1, 0), qt_half(2, 1), "p1")
        for cc in range(4):
            enq_grp((1, 0), kt_cc(2, cc), "p1")
        enq_grp((2, 0), qt_half(3, 0), "p2")
        enq_grp((2, 0), qt_half(3, 1), "p2")
        for cc in range(4):
            enq_grp((2, 0), kt_cc(3, cc), "p2")

        # ---- prologue: QT[0] halves + first half of KT[0]; kt copies on
        # DVE so they run concurrently with the gpsimd qt copies.
        def run_grp(parts):
            parts[0]()
            parts[1]()

        run_grp(qt_half(0, 0))
        run_grp(kt_cc(0, 0, copy_eng="scalar"))
        run_grp(qt_half(0, 1))
        enq_grp((0, 1), kt_cc(0, 1), "k01")

        OPP = 250   # partial out-projection item (1 matmul + DVE add)
        for p in range(KC - 1):
            op_gate[f"op{p}"] = f"av{2 * p + 1}"
            for qti in range(QT8):
                enqueue((2 * p + 2, 0), OPP, op_part(qti, p), f"op{p}")

        # ---- main attention stream --------------------------------------
        # exp batches per head: 10 x 1536 cols + 1 x 1024; head 0 starts
        # with two smaller batches so the ACT stream begins sooner.
        std_sizes = [BCOLS] * 10 + [1024]
        h0_sizes = [512, 512, 1024] + [BCOLS] * 9 + [512]

        for h in range(H):
            dc, hh = h // 2, h % 2
            ppt = pp[h % 3]
            col0 = 0
            sizes = h0_sizes if h == 0 else std_sizes
            if h in (2, 4, 6):
                # this pair's KT/QT projections MUST be emitted before any
                # score matmul that reads them (emission order = dep order)
                drain({f"p{h // 2 - 1}"})
            if h >= 3:
                # AV of head h-3 must be emitted before exp(h) rewrites its
                # pp ring slot (and the V tiles it reads before that)
                if emitted["v"] < 2 * TC:
                    drain({"v"})
                drain({f"av{h - 3}"})
            for bi, ncols in enumerate(sizes):
                if h == 0 and bi == 2:
                    drain({"k01"})
                if h == 0 and bi == 6:
                    drain({"k0late"})
                ps = psSc.tile([128, ncols], F32, tag="psSc", name="pss")
                for m in range(ncols // 512):
                    col = col0 + m * 512
                    c, joff = col // QS, col % QS
                    nc.tensor.matmul(
                        ps[:, m * 512:(m + 1) * 512],
                        kt[dc][hh * DH:hh * DH + DH, c * 128:(c + 1) * 128],
                        qt[dc][hh * DH:hh * DH + DH, joff:joff + 512],
                        start=True, stop=True)
                nc.scalar.activation(
                    out=ppt[:, col0:col0 + ncols], in_=ps,
                    func=mybir.ActivationFunctionType.Exp, scale=SCALE)
                col0 += ncols
                pump((h, bi), 900)
            if h < H - 1:
                for qti in range(QT8):
                    enqueue((h + 1, 3), AVN, av_norm(h, qti), f"av{h}")

        # ---- tail: drain queue, head-7 AV, out projection ---------------
        leftover = sum(1 for it in work)
        if leftover:
            import sys
            print(f"[kernel] {leftover} bg items left to tail",
                  file=sys.stderr)
        pump((H, 0), 10 ** 9)
        for qti in range(QT8):
            # alternate PSUM pools -> 4-deep AV pipelining in the tail;
            # final out-projections trail 4 q-tiles behind so their psum
            # inputs (norm -> PE transpose -> ACT copy) are surely ready.
            av_norm(7, qti, pool=(psA if qti % 2 == 0 else psSc))()
            if qti >= 4:
                outproj_final(qti - 4)()

        def store_half(half):
            # out rows (qti*128 + p) from obig[p, qti, d], 4 q-tiles at a
            # time: DRAM AP iterated (p, qti, d) to match the SBUF source.
            o_ap = bass.AP(
                tensor=out.tensor,
                offset=out.offset + half * (QT8 // 2) * 128 * D,
                ap=[[D, 128], [128 * D, QT8 // 2], [1, D]])
            nc.sync.dma_start(
                out=o_ap,
                in_=obig[:, half * (QT8 // 2):(half + 1) * (QT8 // 2), :])

        store_half(0)
        for qti in range(QT8 - 4, QT8):
            outproj_final(qti)()
        store_half(1)

    nc.compile()
    return nc


def _prep_host(hidden_states, Wq, Wk, Wv, Wo, bo):
    hidden_states = np.asarray(hidden_states, dtype=np.float32)
    w_cat = np.concatenate(
        [np.asarray(a, dtype=np.float32) for a in (Wq, Wk, Wv, Wo)],
        axis=1).astype(ml_dtypes.bfloat16)
    bo = np.asarray(bo, dtype=np.float32)
    xT = [np.ascontiguousarray(hidden_states[b].T).astype(ml_dtypes.bfloat16)
          for b in range(B)]
    return xT, w_cat, bo


def kernel(hidden_states, Wq, Wk, Wv, Wo, bo):
    global LAST_RESULTS
    xT, w_cat, bo = _prep_host(hidden_states, Wq, Wk, Wv, Wo, bo)

    if "nc" not in _CACHE:
        _CACHE["nc"] = _build()
    nc = _CACHE["nc"]

    # odd cores: token halves swapped so queries are always the first QS
    # columns (attention is permutation-invariant over keys; K and V
    # permute together)
    xT_sw = [np.ascontiguousarray(
        np.concatenate([t[:, QS:], t[:, :QS]], axis=1)) for t in xT]
    in_maps = []
    for c in range(NCORES):
        b, qh = c // 2, c % 2
        in_maps.append({
            "x": xT[b] if qh == 0 else xT_sw[b],
            "w": w_cat, "bo": bo,
        })
    res = run_bass_kernel_spmd(nc, in_maps, core_ids=list(range(NCORES)))
    LAST_RESULTS = res

    out = np.empty((B, S, D), dtype=np.float32)
    for c in range(NCORES):
        b, qh = c // 2, c % 2
        out[b, qh * QS:(qh + 1) * QS, :] = res.results[c]["out"].astype(
            np.float32)
    return out
